# revision 25
# baseline (speedup 1.0000x reference)
"""Trainium2 Bass kernel for nn_AdaptiveCausalAttention.

Sharding: 8 cores = 2 batches x 4 head-groups (4 heads each). Each core:
  - QKV projection for its heads (x^T as moving operand, weights as lhsT)
  - banded causal attention: mask[h,i,j] depends only on rel=i-j (Toeplitz),
    materialized once per head as a [128, MW] SBUF buffer whose column slices
    ARE every score tile's mask patch; tiles fully outside the band (mask==0)
    are skipped
  - softmax without max-subtraction (logits are small; the max cancels
    mathematically): e = exp(s/8) * m; denominator via a ones-row in V
  - partial c_proj over its heads' 256 dims
Host sums the 4 partials per batch; no device collectives.

All matmuls in bf16 (f32 PSUM accumulate). float32r is used only for the
reciprocal -> K=1 broadcast matmul of the softmax denominators (fp32 matmuls
lower to the slow two-pass FP32-HIGH mode on this stack, so they are avoided
for bulk work). The 16 per-(head, q-block) denominator rows are batched into
ONE DVE reciprocal (DVE reciprocal time scales with free size only, so a
[16, 512] recip costs the same as a [1, 512] one).
"""
import math

import numpy as np
import ml_dtypes

N_HEAD = 16
N_EMBD = 1024
T = 2048
B = 2
D = 64
R = 32.0
MAX_SPAN = 2048.0
PERIOD_MIN, PERIOD_MAX = 2.0, 4.0
RATIO_MIN, RATIO_MAX = -0.25, 0.25
SPAN_REG = 2e-06

HL = 4          # heads per core
N_CORES = 8
TQ = 512        # query block (matmul moving dim)
TK = 128        # key tile (partition dim)
NQB = T // TQ
DELTA_OFF = TQ - TK  # 384: -min(i0-j0)

TRACE = False
LAST_EXEC_NS = None
LAST_RESULTS = None

_cache = {}


def _sigmoid(x):
    return 1.0 / (1.0 + np.exp(-x.astype(np.float64)))


def _band_tiles(L):
    """Per query-block tuple of key-tile starts j0 intersecting the causal band
    rel in [0, L-1]."""
    tiles = []
    for qi in range(NQB):
        i0 = qi * TQ
        j_hi = min(i0 + TQ - TK, T - TK)
        j_lo = max(0, math.ceil((i0 - L - (TK - 2)) / TK) * TK)
        tiles.append(tuple(range(j_lo, j_hi + 1, TK)))
    return tuple(tiles)


def _build(tiles, mw, zero_bias):
    import concourse.bass as bass
    import concourse.mybir as mybir
    import concourse.tile as tile
    from concourse import bacc

    F32 = mybir.dt.float32
    F32R = mybir.dt.float32r
    BF16 = mybir.dt.bfloat16
    AF = mybir.ActivationFunctionType

    nc = bacc.Bacc(None, target_bir_lowering=False)

    xT_e = nc.dram_tensor("xT", [8, 128, T], BF16, kind="ExternalInput")
    wqk_e = nc.dram_tensor("wqk", [8, 128, 4, 128], BF16, kind="ExternalInput")
    wv_e = nc.dram_tensor("wv", [8, 128, 256], BF16, kind="ExternalInput")
    bqk_e = nc.dram_tensor("bqk", [128, 4], F32, kind="ExternalInput")
    bv_e = nc.dram_tensor("bv", [1, 256], BF16, kind="ExternalInput")
    wp_e = nc.dram_tensor("wp", [2, 128, N_EMBD], BF16, kind="ExternalInput")
    mask_e = nc.dram_tensor("mask", [128, HL, mw], BF16, kind="ExternalInput")
    ones_e = nc.dram_tensor("ones1", [1, 128], BF16, kind="ExternalInput")
    ones65_e = nc.dram_tensor("ones65", [65, 64], F32R, kind="ExternalInput")
    out_e = nc.dram_tensor("out", [T, N_EMBD], F32, kind="ExternalOutput")

    with tile.TileContext(nc) as tc:
        with tc.tile_pool(name="persist", bufs=1) as pp, \
             tc.tile_pool(name="xw", bufs=1) as xp:
            qk_sb = pp.tile([128, 4, T], BF16)       # m-tiles: qA qB kA kB
            v_sb = pp.tile([128, 16, HL, 66], BF16)  # [T-tile, head, D+1+pad]
            mask_sb = pp.tile([128, HL, mw], BF16)
            yT_sb = pp.tile([128, 2, T], BF16)       # chunk c: heads 2c,2c+1
            yTo_sb = pp.tile([64, 2, NQB, TQ], BF16)  # odd heads staging
            wp_sb = pp.tile([128, 2, N_EMBD], BF16)
            bqk_sb = pp.tile([128, 4], F32)
            bv_sb = pp.tile([1, 256], BF16)
            ones_sb = pp.tile([1, 128], BF16)
            ones65_sb = pp.tile([65, 64], F32R)
            x_sb = xp.tile([128, 8, T], BF16)
            wqk_sb = xp.tile([128, 8, 4, 128], BF16)
            wv_sb = xp.tile([128, 8, 256], BF16)

            # critical-path DMAs on sync HWDGE first: x0 + all wqk chunks
            nc.sync.dma_start(x_sb[:, 0, :], xT_e[0])
            for k in range(8):
                nc.sync.dma_start(wqk_sb[:, k, :, :], wqk_e[k])
            nc.sync.dma_start(bqk_sb[:], bqk_e[:])
            for k in range(1, 8):
                nc.sync.dma_start(x_sb[:, k, :], xT_e[k])
            for k in range(8):
                nc.gpsimd.dma_start(wv_sb[:, k, :], wv_e[k])
            nc.gpsimd.dma_start(bv_sb[:], bv_e[:])
            nc.gpsimd.dma_start(ones_sb[:], ones_e[:])
            nc.gpsimd.dma_start(ones65_sb[:], ones65_e[:])
            nc.gpsimd.dma_start(mask_sb[:], mask_e[:])
            nc.gpsimd.dma_start(wp_sb[:, 0, :], wp_e[0])
            nc.gpsimd.dma_start(wp_sb[:, 1, :], wp_e[1])
            nc.vector.memset(v_sb[:, :, :, 64], 1.0)

            with tc.tile_pool(name="att", bufs=22) as ab, \
                 tc.tile_pool(name="attn", bufs=2) as nb, \
                 tc.tile_pool(name="proj", bufs=4) as ob_, \
                 tc.tile_pool(name="ps_pc", bufs=2, space="PSUM") as pspc, \
                 tc.tile_pool(name="ps_s", bufs=2, space="PSUM") as pss, \
                 tc.tile_pool(name="ps_y", bufs=2, space="PSUM") as psy:

                def emit_qk_tile(m, n):
                    ps = pspc.tile([128, 512], F32, tag="pcs")
                    for k in range(8):
                        nc.tensor.matmul(
                            ps[:], wqk_sb[:, k, m, :],
                            x_sb[:, k, n * 512:(n + 1) * 512],
                            start=(k == 0), stop=(k == 7))
                    nc.vector.tensor_scalar_add(
                        qk_sb[:, m, n * 512:(n + 1) * 512], ps[:],
                        bqk_sb[:, m:m + 1])

                def emit_v_tile(t):
                    ps = pspc.tile([128, 256], F32, tag="pcs")
                    for k in range(8):
                        nc.tensor.matmul(
                            ps[:], x_sb[:, k, t * 128:(t + 1) * 128],
                            wv_sb[:, k, :], start=(k == 0),
                            stop=(zero_bias and k == 7))
                    if not zero_bias:
                        nc.tensor.matmul(ps[:], ones_sb[0:1, 0:128],
                                         bv_sb[:], start=False, stop=True)
                    nc.vector.tensor_copy(
                        v_sb[:, t, :, 0:64],
                        ps[:].rearrange("p (h d) -> p h d", h=HL))

                def emit_scores(hp, qi):
                    i0 = qi * TQ
                    es = []
                    for j0 in tiles[qi]:
                        ps_w = pss.tile([128, 1024], F32, tag="pss")
                        nc.tensor.matmul(
                            ps_w[:, 0:512],
                            qk_sb[0:64, 2 + hp, j0:j0 + 128],
                            qk_sb[0:64, hp, i0:i0 + 512],
                            start=True, stop=True)
                        nc.tensor.matmul(
                            ps_w[:, 512:1024],
                            qk_sb[64:128, 2 + hp, j0:j0 + 128],
                            qk_sb[64:128, hp, i0:i0 + 512],
                            start=True, stop=True)
                        e = ab.tile([128, 1024], BF16, tag="e")
                        nc.scalar.activation(e[:], ps_w[:], AF.Exp,
                                             scale=0.125)
                        c0 = (i0 - j0) + DELTA_OFF
                        base = mask_sb[:, 2 * hp, c0:c0 + 512]
                        mk = bass.AP(base.tensor, base.offset,
                                     [list(base.ap[0]), [mw, 2], [1, 512]])
                        nc.vector.tensor_mul(
                            e[:].rearrange("p (two n) -> p two n", two=2),
                            e[:].rearrange("p (two n) -> p two n", two=2),
                            mk)
                        es.append((j0, e))
                    return es

                def emit_attv(hp, qi, es):
                    pys = []
                    for hh in range(2):
                        h = 2 * hp + hh
                        ps_y = psy.tile([65, 512], F32, tag="psy")
                        for i, (j0, e) in enumerate(es):
                            nc.tensor.matmul(
                                ps_y[0:65, :], v_sb[:, j0 // 128, h, 0:65],
                                e[:, hh * 512:(hh + 1) * 512],
                                start=(i == 0), stop=(i == len(es) - 1))
                        pys.append(ps_y)
                    return pys

                def emit_unit_tail(hp, qi, pys, den_t):
                    i0 = qi * TQ
                    for hh in range(2):
                        ps_y = pys[hh]
                        row = (2 * hp + hh) * TQ
                        nc.vector.tensor_copy(den_t[64:65, row:row + TQ],
                                              ps_y[64:65, :])
                        if hh == 0:
                            sl = yT_sb[0:64, hp, i0:i0 + 512]
                        else:
                            sl = yTo_sb[:, hp, qi, :]
                        nc.vector.tensor_copy(sl, ps_y[0:64, :])

                def emit_qi_norm(qi, den_t):
                    i0 = qi * TQ
                    nc.scalar.activation(den_t[64:65, :], den_t[64:65, :],
                                         AF.Ln)
                    with nc.allow_low_precision(reason="softmax recip"):
                        nc.scalar.activation(
                            den_t[64:65, :], den_t[64:65, :], AF.Exp,
                            scale=-1.0)
                    for hp in range(2):
                        for hh in range(2):
                            row = 2 * hp + hh
                            ps_bb = pspc.tile([64, 512], F32, tag="pcs")
                            nc.tensor.matmul(
                                ps_bb[:], ones65_sb[64:65, :],
                                den_t[64:65, row * TQ:(row + 1) * TQ],
                                start=True, stop=True)
                            if hh == 0:
                                sl = yT_sb[0:64, hp, i0:i0 + 512]
                                nc.vector.tensor_mul(sl, sl, ps_bb[:])
                            else:
                                sl = yTo_sb[:, hp, qi, :]
                                nc.vector.tensor_mul(sl, sl, ps_bb[:])
                                nc.sync.dma_start(
                                    yT_sb[64:128, hp, i0:i0 + 512], sl)

                def emit_proj_qi(qi):
                    for t in range(4 * qi, 4 * qi + 4):
                        for cb in range(2):
                            ps_o = pspc.tile([128, 512], F32, tag="pcs")
                            for c in range(2):
                                nc.tensor.matmul(
                                    ps_o[:],
                                    yT_sb[:, c, t * 128:(t + 1) * 128],
                                    wp_sb[:, c, cb * 512:(cb + 1) * 512],
                                    start=(c == 0), stop=(c == 1))
                            ob = ob_.tile([128, 512], F32, tag="ob")
                            nc.vector.tensor_copy(ob[:], ps_o[:])
                            nc.sync.dma_start(
                                out_e[t * 128:(t + 1) * 128,
                                      cb * 512:(cb + 1) * 512], ob[:])

                pre_pieces = {
                    (0, 0): [("qk", 0, 0), ("qk", 2, 0)],
                }
                post_pieces = {
                    (0, 0): [("qk", 1, 0), ("qk", 3, 0), ("v", 0), ("v", 1)],
                    (1, 0): [("v", 2), ("v", 3), ("qk", 0, 1), ("qk", 2, 1)],
                    (0, 1): [("qk", 1, 1), ("qk", 3, 1), ("v", 4), ("v", 5)],
                    (1, 1): [("v", 6), ("v", 7), ("qk", 0, 2), ("qk", 2, 2)],
                    (0, 2): [("qk", 1, 2), ("qk", 3, 2), ("v", 8), ("v", 9)],
                    (1, 2): [("v", 10), ("v", 11), ("qk", 0, 3),
                             ("qk", 2, 3)],
                    (0, 3): [("qk", 1, 3), ("qk", 3, 3), ("v", 12),
                             ("v", 13)],
                    (1, 3): [("v", 14), ("v", 15)],
                }

                def emit_pieces(lst):
                    for pc in lst:
                        if pc[0] == "qk":
                            emit_qk_tile(pc[1], pc[2])
                        else:
                            emit_v_tile(pc[1])

                units = [(hp, qi) for qi in range(NQB) for hp in range(2)]
                pend = None
                den_tiles = {}
                for hp, qi in units:
                    emit_pieces(pre_pieces.get((hp, qi), []))
                    if hp == 0:
                        den_q_t = nb.tile([65, 4 * TQ], F32R, tag="den")
                        den_tiles[qi] = den_q_t
                    es = emit_scores(hp, qi)
                    emit_pieces(post_pieces.get((hp, qi), []))
                    if pend is not None:
                        php, pqi, pes = pend
                        pys = emit_attv(php, pqi, pes)
                        emit_unit_tail(php, pqi, pys, den_tiles[pqi])
                        if php == 1:
                            emit_qi_norm(pqi, den_tiles[pqi])
                            emit_proj_qi(pqi)
                    pend = (hp, qi, es)
                php, pqi, pes = pend
                pys = emit_attv(php, pqi, pes)
                emit_unit_tail(php, pqi, pys, den_tiles[pqi])
                emit_qi_norm(pqi, den_tiles[pqi])
                emit_proj_qi(pqi)
    nc.compile()
    return nc


def _get_nc(tiles, mw, zero_bias):
    key = (tiles, mw, zero_bias)
    if key not in _cache:
        _cache[key] = _build(tiles, mw, zero_bias)
    return _cache[key]


def _bf16(a):
    return np.ascontiguousarray(a.astype(ml_dtypes.bfloat16))


def kernel(x, w_attn, b_attn, w_proj, b_proj, span_params, period_weight,
           ratio_weight):
    global LAST_EXEC_NS, LAST_RESULTS
    from concourse.bass_utils import run_bass_kernel_spmd

    x = np.ascontiguousarray(np.asarray(x, np.float32))
    w_attn = np.ascontiguousarray(np.asarray(w_attn, np.float32))
    b_attn = np.asarray(b_attn, np.float32)
    w_proj = np.ascontiguousarray(np.asarray(w_proj, np.float32))
    b_proj = np.asarray(b_proj, np.float32)
    span_params = np.asarray(span_params, np.float32)
    period_weight = np.asarray(period_weight, np.float32)
    ratio_weight = np.asarray(ratio_weight, np.float32)

    # per-head mask tables m_h(rel), rel in [0, T)
    spans = (_sigmoid(span_params) * MAX_SPAN).astype(np.float64)      # [16]
    period = PERIOD_MIN + (PERIOD_MAX - PERIOD_MIN) * _sigmoid(period_weight)
    ratio = RATIO_MIN + (RATIO_MAX - RATIO_MIN) * _sigmoid(ratio_weight)
    amp = period / 4.0
    off = period * ratio
    rel = np.arange(T, dtype=np.float64)
    mask_pos = np.clip((R - rel[None, :] + spans[:, None]) / R, 0.0, 1.0)
    wave = np.clip(0.5 * (np.cos(2.0 * math.pi * rel[None, :]
                                 / period[:, None]) + 1.0) * amp[:, None]
                   + 0.5 + off[:, None], 0.0, 1.0)
    mtab = (mask_pos * wave).astype(np.float32)                        # [16,T]

    L = int(min(T, math.ceil(spans.max() + R)))
    tiles = _band_tiles(L)
    max_delta = max(qi * TQ - t[0] for qi, t in enumerate(tiles))
    mw = max_delta + DELTA_OFF + TQ + TK  # +TK slack for the paired-AP window
    zero_bias = not np.any(b_attn)
    nc = _get_nc(tiles, mw, zero_bias)

    # mask wide buffers: W[p, h, u] = m_h(u - DELTA_OFF - p), 0 outside [0,T)
    p_idx = np.arange(128)
    u_idx = np.arange(mw)
    ridx = u_idx[None, :] - DELTA_OFF - p_idx[:, None]                 # [128,mw]
    valid = (ridx >= 0) & (ridx < T)
    ridx_c = np.clip(ridx, 0, T - 1)

    in_maps = []
    for c in range(N_CORES):
        b = c // 4
        g = c % 4
        h0 = g * HL
        cs = slice(256 * g, 256 * g + 256)
        xT = _bf16(np.ascontiguousarray(x[b].T)).reshape(8, 128, T)
        wqk = _bf16(np.concatenate(
            [w_attn[:, cs],
             w_attn[:, 1024 + 256 * g: 1024 + 256 * g + 256]], 1)
        ).reshape(8, 128, 4, 128)
        wv = _bf16(w_attn[:, 2048 + 256 * g: 2048 + 256 * g + 256]
                   ).reshape(8, 128, 256)
        bqk = np.ascontiguousarray(
            np.concatenate([b_attn[cs], b_attn[1024 + 256 * g:
                                               1024 + 256 * g + 256]])
            .reshape(4, 128).T)
        bv = _bf16(b_attn[2048 + 256 * g: 2048 + 256 * g + 256].reshape(1, 256))
        wp = _bf16(w_proj[cs, :]).reshape(2, 128, N_EMBD)
        mk = np.where(valid[:, None, :],
                      mtab[h0:h0 + HL][:, ridx_c].transpose(1, 0, 2), 0.0)
        in_maps.append({
            "xT": xT, "wqk": wqk, "wv": wv, "bqk": bqk, "bv": bv, "wp": wp,
            "mask": _bf16(mk),
            "ones1": _bf16(np.ones((1, 128), np.float32)),
            "ones65": np.ones((65, 64), np.float32),
        })

    res = run_bass_kernel_spmd(nc, in_maps, core_ids=list(range(N_CORES)),
                               trace=TRACE)
    LAST_EXEC_NS = res.exec_time_ns
    LAST_RESULTS = res

    y = np.zeros((B, T, N_EMBD), np.float32)
    for c in range(N_CORES):
        y[c // 4] += res.results[c]["out"]
    y += b_proj[None, None, :]

    loss_terms = (1.0 / period + 2.0 * ratio - 1.0 / PERIOD_MAX
                  - 2.0 * RATIO_MIN)
    span_loss = np.float32(SPAN_REG * np.sum((spans + R) * loss_terms)
                           / N_HEAD)
    return y, span_loss


# revision 26
# speedup vs baseline: 1.0617x; 1.0617x over previous
"""Trainium2 Bass kernel for nn_AdaptiveCausalAttention.

Sharding: 8 cores = 2 batches x 4 head-groups (4 heads each). Each core:
  - QKV projection for its heads (x^T as moving operand, weights as lhsT)
  - banded causal attention: mask[h,i,j] depends only on rel=i-j (Toeplitz),
    materialized once per head as a [128, MW] SBUF buffer whose column slices
    ARE every score tile's mask patch; tiles fully outside the band (mask==0)
    are skipped
  - softmax without max-subtraction (logits are small; the max cancels
    mathematically): e = exp(s/8) * m; denominator via a ones-row in V
  - partial c_proj over its heads' 256 dims
Host sums the 4 partials per batch; no device collectives.

All matmuls in bf16 (f32 PSUM accumulate). float32r is used only for the
reciprocal -> K=1 broadcast matmul of the softmax denominators (fp32 matmuls
lower to the slow two-pass FP32-HIGH mode on this stack, so they are avoided
for bulk work). The 16 per-(head, q-block) denominator rows are batched into
ONE DVE reciprocal (DVE reciprocal time scales with free size only, so a
[16, 512] recip costs the same as a [1, 512] one).
"""
import math

import numpy as np
import ml_dtypes

N_HEAD = 16
N_EMBD = 1024
T = 2048
B = 2
D = 64
R = 32.0
MAX_SPAN = 2048.0
PERIOD_MIN, PERIOD_MAX = 2.0, 4.0
RATIO_MIN, RATIO_MAX = -0.25, 0.25
SPAN_REG = 2e-06

HL = 4          # heads per core
N_CORES = 8
TQ = 512        # query block (matmul moving dim)
TK = 128        # key tile (partition dim)
NQB = T // TQ
DELTA_OFF = TQ - TK  # 384: -min(i0-j0)

TRACE = False
LAST_EXEC_NS = None
LAST_RESULTS = None

_cache = {}


def _sigmoid(x):
    return 1.0 / (1.0 + np.exp(-x.astype(np.float64)))


def _band_tiles(L):
    """Per query-block tuple of key-tile starts j0 intersecting the causal band
    rel in [0, L-1]."""
    tiles = []
    for qi in range(NQB):
        i0 = qi * TQ
        j_hi = min(i0 + TQ - TK, T - TK)
        j_lo = max(0, math.ceil((i0 - L - (TK - 2)) / TK) * TK)
        tiles.append(tuple(range(j_lo, j_hi + 1, TK)))
    return tuple(tiles)


def _build(tiles, mw, zero_bias):
    import concourse.bass as bass
    import concourse.mybir as mybir
    import concourse.tile as tile
    from concourse import bacc

    F32 = mybir.dt.float32
    F32R = mybir.dt.float32r
    BF16 = mybir.dt.bfloat16
    AF = mybir.ActivationFunctionType

    nc = bacc.Bacc(None, target_bir_lowering=False)

    xT_e = nc.dram_tensor("xT", [8, 128, T], BF16, kind="ExternalInput")
    wqk_e = nc.dram_tensor("wqk", [8, 128, 4, 128], BF16, kind="ExternalInput")
    wv_e = nc.dram_tensor("wv", [8, 128, 256], BF16, kind="ExternalInput")
    bqk_e = nc.dram_tensor("bqk", [128, 4], F32, kind="ExternalInput")
    bv_e = nc.dram_tensor("bv", [1, 256], BF16, kind="ExternalInput")
    wp_e = nc.dram_tensor("wp", [2, 128, N_EMBD], BF16, kind="ExternalInput")
    mask_e = nc.dram_tensor("mask", [128, HL, mw], BF16, kind="ExternalInput")
    ones_e = nc.dram_tensor("ones1", [1, 128], BF16, kind="ExternalInput")
    ones65_e = nc.dram_tensor("ones65", [65, 64], F32R, kind="ExternalInput")
    out_e = nc.dram_tensor("out", [T, N_EMBD], F32, kind="ExternalOutput")

    with tile.TileContext(nc) as tc:
        with tc.tile_pool(name="persist", bufs=1) as pp, \
             tc.tile_pool(name="xw", bufs=1) as xp:
            qk_sb = pp.tile([128, 4, T], BF16)       # m-tiles: qA qB kA kB
            v_sb = pp.tile([128, 16, HL, 66], BF16)  # [T-tile, head, D+1+pad]
            mask_sb = pp.tile([128, HL, mw], BF16)
            yT_sb = pp.tile([128, 2, T], BF16)       # chunk c: heads 2c,2c+1
            yTo_sb = pp.tile([64, 2, NQB, TQ], BF16)  # odd heads staging
            wp_sb = pp.tile([128, 2, N_EMBD], BF16)
            bqk_sb = pp.tile([128, 4], F32)
            bv_sb = pp.tile([1, 256], BF16)
            ones_sb = pp.tile([1, 128], BF16)
            ones65_sb = pp.tile([65, 64], F32R)
            x_sb = xp.tile([128, 8, T], BF16)
            wqk_sb = xp.tile([128, 8, 4, 128], BF16)
            wv_sb = xp.tile([128, 8, 256], BF16)

            # critical-path DMAs on sync HWDGE first: x0 + all wqk chunks
            nc.sync.dma_start(x_sb[:, 0, :], xT_e[0])
            for k in range(8):
                nc.sync.dma_start(wqk_sb[:, k, :, :], wqk_e[k])
            nc.sync.dma_start(bqk_sb[:], bqk_e[:])
            for k in range(1, 8):
                nc.sync.dma_start(x_sb[:, k, :], xT_e[k])
            for k in range(8):
                nc.gpsimd.dma_start(wv_sb[:, k, :], wv_e[k])
            nc.gpsimd.dma_start(bv_sb[:], bv_e[:])
            nc.gpsimd.dma_start(ones_sb[:], ones_e[:])
            nc.gpsimd.dma_start(ones65_sb[:], ones65_e[:])
            nc.gpsimd.dma_start(mask_sb[:], mask_e[:])
            nc.gpsimd.dma_start(wp_sb[:, 0, :], wp_e[0])
            nc.gpsimd.dma_start(wp_sb[:, 1, :], wp_e[1])
            nc.vector.memset(v_sb[:, :, :, 64], 1.0)

            with tc.tile_pool(name="att", bufs=22) as ab, \
                 tc.tile_pool(name="attn", bufs=2) as nb, \
                 tc.tile_pool(name="proj", bufs=4) as ob_, \
                 tc.tile_pool(name="ps_pc", bufs=2, space="PSUM") as pspc, \
                 tc.tile_pool(name="ps_s", bufs=2, space="PSUM") as pss, \
                 tc.tile_pool(name="ps_y", bufs=2, space="PSUM") as psy:

                def emit_qk_tile(m, n):
                    ps = pspc.tile([128, 512], F32, tag="pcs")
                    for k in range(8):
                        nc.tensor.matmul(
                            ps[:], wqk_sb[:, k, m, :],
                            x_sb[:, k, n * 512:(n + 1) * 512],
                            start=(k == 0), stop=(k == 7))
                    nc.vector.tensor_scalar_add(
                        qk_sb[:, m, n * 512:(n + 1) * 512], ps[:],
                        bqk_sb[:, m:m + 1])

                def emit_v_tile(t):
                    ps = pspc.tile([128, 256], F32, tag="pcs")
                    for k in range(8):
                        nc.tensor.matmul(
                            ps[:], x_sb[:, k, t * 128:(t + 1) * 128],
                            wv_sb[:, k, :], start=(k == 0),
                            stop=(zero_bias and k == 7))
                    if not zero_bias:
                        nc.tensor.matmul(ps[:], ones_sb[0:1, 0:128],
                                         bv_sb[:], start=False, stop=True)
                    nc.vector.tensor_copy(
                        v_sb[:, t, :, 0:64],
                        ps[:].rearrange("p (h d) -> p h d", h=HL))

                def emit_scores(hp, qi):
                    i0 = qi * TQ
                    es = []
                    for j0 in tiles[qi]:
                        ps_w = pss.tile([128, 1024], F32, tag="pss")
                        nc.tensor.matmul(
                            ps_w[:, 0:512],
                            qk_sb[0:64, 2 + hp, j0:j0 + 128],
                            qk_sb[0:64, hp, i0:i0 + 512],
                            start=True, stop=True)
                        nc.tensor.matmul(
                            ps_w[:, 512:1024],
                            qk_sb[64:128, 2 + hp, j0:j0 + 128],
                            qk_sb[64:128, hp, i0:i0 + 512],
                            start=True, stop=True)
                        e = ab.tile([128, 1024], BF16, tag="e")
                        nc.scalar.activation(e[:], ps_w[:], AF.Exp,
                                             scale=0.125)
                        c0 = (i0 - j0) + DELTA_OFF
                        base = mask_sb[:, 2 * hp, c0:c0 + 512]
                        mk = bass.AP(base.tensor, base.offset,
                                     [list(base.ap[0]), [mw, 2], [1, 512]])
                        nc.vector.tensor_mul(
                            e[:].rearrange("p (two n) -> p two n", two=2),
                            e[:].rearrange("p (two n) -> p two n", two=2),
                            mk)
                        es.append((j0, e))
                    return es

                def emit_attv(hp, qi, es):
                    pys = []
                    for hh in range(2):
                        h = 2 * hp + hh
                        ps_y = psy.tile([65, 512], F32, tag="psy")
                        for i, (j0, e) in enumerate(es):
                            nc.tensor.matmul(
                                ps_y[0:65, :], v_sb[:, j0 // 128, h, 0:65],
                                e[:, hh * 512:(hh + 1) * 512],
                                start=(i == 0), stop=(i == len(es) - 1))
                        pys.append(ps_y)
                    return pys

                def emit_unit_tail(hp, qi, pys, den_t):
                    i0 = qi * TQ
                    for hh in range(2):
                        ps_y = pys[hh]
                        row = (2 * hp + hh) * TQ
                        nc.scalar.activation(den_t[64:65, row:row + TQ],
                                             ps_y[64:65, :], AF.Ln)
                        if hh == 0:
                            sl = yT_sb[0:64, hp, i0:i0 + 512]
                        else:
                            sl = yTo_sb[:, hp, qi, :]
                        nc.vector.tensor_copy(sl, ps_y[0:64, :])

                def emit_qi_norm(qi, den_t):
                    i0 = qi * TQ
                    with nc.allow_low_precision(reason="softmax recip"):
                        nc.scalar.activation(
                            den_t[64:65, :], den_t[64:65, :], AF.Exp,
                            scale=-1.0)
                    for hp in range(2):
                        for hh in range(2):
                            row = 2 * hp + hh
                            ps_bb = pspc.tile([64, 512], F32, tag="pcs")
                            nc.tensor.matmul(
                                ps_bb[:], ones65_sb[64:65, :],
                                den_t[64:65, row * TQ:(row + 1) * TQ],
                                start=True, stop=True)
                            if hh == 0:
                                sl = yT_sb[0:64, hp, i0:i0 + 512]
                                nc.vector.tensor_mul(sl, sl, ps_bb[:])
                            else:
                                sl = yTo_sb[:, hp, qi, :]
                                nc.vector.tensor_mul(sl, sl, ps_bb[:])
                                nc.sync.dma_start(
                                    yT_sb[64:128, hp, i0:i0 + 512], sl)

                def emit_proj_qi(qi):
                    for t in range(4 * qi, 4 * qi + 4):
                        for cb in range(2):
                            ps_o = pspc.tile([128, 512], F32, tag="pcs")
                            for c in range(2):
                                nc.tensor.matmul(
                                    ps_o[:],
                                    yT_sb[:, c, t * 128:(t + 1) * 128],
                                    wp_sb[:, c, cb * 512:(cb + 1) * 512],
                                    start=(c == 0), stop=(c == 1))
                            ob = ob_.tile([128, 512], F32, tag="ob")
                            nc.vector.tensor_copy(ob[:], ps_o[:])
                            nc.sync.dma_start(
                                out_e[t * 128:(t + 1) * 128,
                                      cb * 512:(cb + 1) * 512], ob[:])

                pre_pieces = {
                    (0, 0): [("qk", 0, 0), ("qk", 2, 0)],
                }
                post_pieces = {
                    (0, 0): [("qk", 1, 0), ("qk", 3, 0), ("v", 0), ("v", 1)],
                    (1, 0): [("v", 2), ("v", 3), ("qk", 0, 1), ("qk", 2, 1)],
                    (0, 1): [("qk", 1, 1), ("qk", 3, 1), ("v", 4), ("v", 5)],
                    (1, 1): [("v", 6), ("v", 7), ("qk", 0, 2), ("qk", 2, 2)],
                    (0, 2): [("qk", 1, 2), ("qk", 3, 2), ("v", 8), ("v", 9)],
                    (1, 2): [("v", 10), ("v", 11), ("qk", 0, 3),
                             ("qk", 2, 3)],
                    (0, 3): [("qk", 1, 3), ("qk", 3, 3), ("v", 12),
                             ("v", 13)],
                    (1, 3): [("v", 14), ("v", 15)],
                }

                def emit_pieces(lst):
                    for pc in lst:
                        if pc[0] == "qk":
                            emit_qk_tile(pc[1], pc[2])
                        else:
                            emit_v_tile(pc[1])

                units = [(hp, qi) for qi in range(NQB) for hp in range(2)]
                pend = None
                den_tiles = {}
                for hp, qi in units:
                    emit_pieces(pre_pieces.get((hp, qi), []))
                    if hp == 0:
                        den_q_t = nb.tile([65, 4 * TQ], F32R, tag="den")
                        den_tiles[qi] = den_q_t
                    es = emit_scores(hp, qi)
                    emit_pieces(post_pieces.get((hp, qi), []))
                    if pend is not None:
                        php, pqi, pes = pend
                        pys = emit_attv(php, pqi, pes)
                        emit_unit_tail(php, pqi, pys, den_tiles[pqi])
                        if php == 1:
                            emit_qi_norm(pqi, den_tiles[pqi])
                            emit_proj_qi(pqi)
                    pend = (hp, qi, es)
                php, pqi, pes = pend
                pys = emit_attv(php, pqi, pes)
                emit_unit_tail(php, pqi, pys, den_tiles[pqi])
                emit_qi_norm(pqi, den_tiles[pqi])
                emit_proj_qi(pqi)
    nc.compile()
    return nc


def _get_nc(tiles, mw, zero_bias):
    key = (tiles, mw, zero_bias)
    if key not in _cache:
        _cache[key] = _build(tiles, mw, zero_bias)
    return _cache[key]


def _bf16(a):
    return np.ascontiguousarray(a.astype(ml_dtypes.bfloat16))


def kernel(x, w_attn, b_attn, w_proj, b_proj, span_params, period_weight,
           ratio_weight):
    global LAST_EXEC_NS, LAST_RESULTS
    from concourse.bass_utils import run_bass_kernel_spmd

    x = np.ascontiguousarray(np.asarray(x, np.float32))
    w_attn = np.ascontiguousarray(np.asarray(w_attn, np.float32))
    b_attn = np.asarray(b_attn, np.float32)
    w_proj = np.ascontiguousarray(np.asarray(w_proj, np.float32))
    b_proj = np.asarray(b_proj, np.float32)
    span_params = np.asarray(span_params, np.float32)
    period_weight = np.asarray(period_weight, np.float32)
    ratio_weight = np.asarray(ratio_weight, np.float32)

    # per-head mask tables m_h(rel), rel in [0, T)
    spans = (_sigmoid(span_params) * MAX_SPAN).astype(np.float64)      # [16]
    period = PERIOD_MIN + (PERIOD_MAX - PERIOD_MIN) * _sigmoid(period_weight)
    ratio = RATIO_MIN + (RATIO_MAX - RATIO_MIN) * _sigmoid(ratio_weight)
    amp = period / 4.0
    off = period * ratio
    rel = np.arange(T, dtype=np.float64)
    mask_pos = np.clip((R - rel[None, :] + spans[:, None]) / R, 0.0, 1.0)
    wave = np.clip(0.5 * (np.cos(2.0 * math.pi * rel[None, :]
                                 / period[:, None]) + 1.0) * amp[:, None]
                   + 0.5 + off[:, None], 0.0, 1.0)
    mtab = (mask_pos * wave).astype(np.float32)                        # [16,T]

    L = int(min(T, math.ceil(spans.max() + R)))
    tiles = _band_tiles(L)
    max_delta = max(qi * TQ - t[0] for qi, t in enumerate(tiles))
    mw = max_delta + DELTA_OFF + TQ + TK  # +TK slack for the paired-AP window
    zero_bias = not np.any(b_attn)
    nc = _get_nc(tiles, mw, zero_bias)

    # mask wide buffers: W[p, h, u] = m_h(u - DELTA_OFF - p), 0 outside [0,T)
    p_idx = np.arange(128)
    u_idx = np.arange(mw)
    ridx = u_idx[None, :] - DELTA_OFF - p_idx[:, None]                 # [128,mw]
    valid = (ridx >= 0) & (ridx < T)
    ridx_c = np.clip(ridx, 0, T - 1)

    in_maps = []
    for c in range(N_CORES):
        b = c // 4
        g = c % 4
        h0 = g * HL
        cs = slice(256 * g, 256 * g + 256)
        xT = _bf16(np.ascontiguousarray(x[b].T)).reshape(8, 128, T)
        wqk = _bf16(np.concatenate(
            [w_attn[:, cs],
             w_attn[:, 1024 + 256 * g: 1024 + 256 * g + 256]], 1)
        ).reshape(8, 128, 4, 128)
        wv = _bf16(w_attn[:, 2048 + 256 * g: 2048 + 256 * g + 256]
                   ).reshape(8, 128, 256)
        bqk = np.ascontiguousarray(
            np.concatenate([b_attn[cs], b_attn[1024 + 256 * g:
                                               1024 + 256 * g + 256]])
            .reshape(4, 128).T)
        bv = _bf16(b_attn[2048 + 256 * g: 2048 + 256 * g + 256].reshape(1, 256))
        wp = _bf16(w_proj[cs, :]).reshape(2, 128, N_EMBD)
        mk = np.where(valid[:, None, :],
                      mtab[h0:h0 + HL][:, ridx_c].transpose(1, 0, 2), 0.0)
        in_maps.append({
            "xT": xT, "wqk": wqk, "wv": wv, "bqk": bqk, "bv": bv, "wp": wp,
            "mask": _bf16(mk),
            "ones1": _bf16(np.ones((1, 128), np.float32)),
            "ones65": np.ones((65, 64), np.float32),
        })

    res = run_bass_kernel_spmd(nc, in_maps, core_ids=list(range(N_CORES)),
                               trace=TRACE)
    LAST_EXEC_NS = res.exec_time_ns
    LAST_RESULTS = res

    y = np.zeros((B, T, N_EMBD), np.float32)
    for c in range(N_CORES):
        y[c // 4] += res.results[c]["out"]
    y += b_proj[None, None, :]

    loss_terms = (1.0 / period + 2.0 * ratio - 1.0 / PERIOD_MAX
                  - 2.0 * RATIO_MIN)
    span_loss = np.float32(SPAN_REG * np.sum((spans + R) * loss_terms)
                           / N_HEAD)
    return y, span_loss
